# revision 22
# baseline (speedup 1.0000x reference)
"""Deformable 1D convolution for Trainium2 (8 NeuronCores, data-parallel over batch).

Math (validated against the reference):
    p[t,k]   = clip(k + offsets[b,0,t,k], 0, 2)
    c[k,j,t] = mask[b,k,t] * relu(1 - |p[t,k] - j|)      j in {0,1,2}
    out[b,o,t] = sum_{k,j} c[k,j,t] * (W_k @ x[b])[o, t+j] + bias[o]

v12 "banded-first": both heavy stages run on the PE.

  Host prepares, per chunk i of CH=126 outputs:
    xT_stag[:, 128i:128(i+1)] = x^T rows 126i..126i+128   (staggered x^T)
    A[p, k*126+t] = c[k, p-t, 126i+t] for p-t in {0,1,2}  (banded coeffs, bf16)

  Per chunk (one 128-contraction matmul each):
    MM-B: xs[c, (k,t)] = xT_blk^T @ A        (378 cols; one stationary for all k)
    copy: xs -> bf16 SBUF, split Act/DVE
  Per group of G=4 chunks (stationary W_k amortized over the group):
    MM-W k=0..2: out'[o, (g,t)] += W_k^T @ xs_k[c, (g,t)]  (strided rhs AP)
    bias add (per-partition scalar, out' is [C_out, t]) + bf16 cast,
    alternating Act(Identity+bias) / DVE(tensor_scalar); out DMA per group
    issued from the otherwise-idle GpSimd SWDGE queue.
  Chunks are processed in PAIRS sharing a 2-bank PSUM tile so each Act/DVE
  copy instruction covers two chunks (halves per-instruction overhead).
  Output layout [C_out, t] => host unshard is a concat.
"""

import numpy as np
import ml_dtypes
from contextlib import ExitStack

import concourse.bass as bass
import concourse.mybir as mybir
import concourse.tile as tile
from concourse import bacc
from concourse import bass_utils
from concourse.ap import AP

F32 = mybir.dt.float32
BF16 = mybir.dt.bfloat16
OP = mybir.AluOpType
ACTF = mybir.ActivationFunctionType

B, C, L, K = 16, 128, 4096, 3
LOUT = L - (K - 1)          # 4094
NCORES = 8
BPC = B // NCORES           # batches per core (2)
CH = 126                    # output positions per chunk
NS = -(-LOUT // CH)         # 33 chunks
AW = K * CH                 # A-tile cols per chunk (378)
XW = NS * 128               # staggered xT cols per batch (4224)
G = 4                       # chunks per W-group (one PSUM bank half)
NG = -(-NS // G)            # 9 groups (8 full + 1 single-chunk)

_CACHE = {}


def _build_program():
    if "nc" in _CACHE:
        return _CACHE["nc"]

    nc = bacc.Bacc(
        "TRN2",
        target_bir_lowering=False,
        debug=False,
        enable_asserts=False,
        num_devices=NCORES,
    )

    xt_in = nc.dram_tensor("xt_in", [BPC, 128, XW], BF16, kind="ExternalInput").ap()
    a_in = nc.dram_tensor("a_in", [BPC, 128, NS * AW], BF16,
                          kind="ExternalInput").ap()
    wt = nc.dram_tensor("wt", [C, K * C], BF16, kind="ExternalInput").ap()
    bvec = nc.dram_tensor("bvec", [128, 1], F32, kind="ExternalInput").ap()
    outT = nc.dram_tensor("outT", [BPC, C, LOUT], BF16, kind="ExternalOutput").ap()

    with tile.TileContext(nc) as tc, ExitStack() as ctx:
        const_pool = ctx.enter_context(tc.tile_pool(name="const", bufs=1))
        xt_pool = ctx.enter_context(tc.tile_pool(name="xt", bufs=2))
        a_pool = ctx.enter_context(tc.tile_pool(name="a", bufs=2))
        xs_pool = ctx.enter_context(tc.tile_pool(name="xs", bufs=3))
        o_pool = ctx.enter_context(tc.tile_pool(name="o", bufs=3))
        xps_pool = ctx.enter_context(tc.tile_pool(name="xps", bufs=2, space="PSUM"))
        ops_pool = ctx.enter_context(tc.tile_pool(name="ops", bufs=2, space="PSUM"))

        wt_sb = const_pool.tile([128, K * C], BF16)
        bias_sb = const_pool.tile([128, 1], F32)

        for b in range(BPC):
            # interleave A/xT slices so chunk 0's inputs land first
            xt_sb = xt_pool.tile([128, XW], BF16)
            a_sb = a_pool.tile([128, NS * AW], BF16)
            xt_splits = ((0, 256), (256, 1024), (1024, 2560), (2560, XW))
            a_splits = ((0, 1), (1, 3), (3, 8), (8, 15), (15, 24), (24, NS))
            order = [("a", a_splits[0]), ("x", xt_splits[0]),
                     ("a", a_splits[1]), ("x", xt_splits[1]),
                     ("a", a_splits[2]), ("x", xt_splits[2]),
                     ("a", a_splits[3]), ("x", xt_splits[3]),
                     ("a", a_splits[4]), ("a", a_splits[5])]
            for di, (kind, (s0, s1)) in enumerate(order):
                if kind == "x":
                    nc.sync.dma_start(xt_sb[:, s0:s1], xt_in[b][:, s0:s1])
                else:
                    nc.sync.dma_start(a_sb[:, s0 * AW:s1 * AW],
                                      a_in[b][:, s0 * AW:s1 * AW])
                if b == 0 and di == 1:
                    # constants needed only by MM-W / casts: issue after the
                    # first critical a/xt slices
                    nc.sync.dma_start(wt_sb[:], wt[:])
                    nc.sync.dma_start(bias_sb[:], bvec[:])

            xs_tiles = {}   # group -> xs_sb tile [128, G*AW]
            NPAIR = -(-NS // 2)          # 17 chunk-pairs

            def stage_b_pair(p):
                """MM-B for chunks 2p, 2p+1 into one 2-bank PSUM tile,
                then one paired Act copy + one paired DVE copy."""
                nn = min(2, NS - 2 * p)
                g = (2 * p) // G
                xps = xps_pool.tile([128, 1024], F32, name="xps", tag="xps")
                for ii in range(nn):
                    i = 2 * p + ii
                    nc.tensor.matmul(xps[0:128, 512 * ii:512 * ii + AW],
                                     xt_sb[:, 128 * i:128 * (i + 1)],
                                     a_sb[:, i * AW:(i + 1) * AW],
                                     start=True, stop=True)
                if g not in xs_tiles:
                    xs_tiles[g] = xs_pool.tile([128, G * AW], BF16,
                                               name="xs", tag="xs")
                xs = xs_tiles[g]
                o0 = ((2 * p) % G) * AW
                # paired copies: src strides 512 (psum slots), dst AW (xs slots)
                nc.scalar.activation(
                    AP(xs.tensor, o0, [[G * AW, 128], [AW, nn], [1, 184]]),
                    AP(xps.tensor, 0, [[1024, 128], [512, nn], [1, 184]]),
                    ACTF.Copy)
                nc.vector.tensor_copy(
                    AP(xs.tensor, o0 + 184, [[G * AW, 128], [AW, nn], [1, 194]]),
                    AP(xps.tensor, 184, [[1024, 128], [512, nn], [1, 194]]))

            ops_tiles = {}

            def stage_w(g):
                """W-contraction for group g into its ops-pair half; on the
                pair's second group (or the last), bias+cast + out DMA."""
                gn = min(G, NS - g * G)
                xs = xs_tiles.pop(g)
                gp = g // 2
                if gp not in ops_tiles:
                    ops_tiles[gp] = ops_pool.tile([128, 1024], F32,
                                                  name="ops", tag="ops")
                ops = ops_tiles[gp]
                o0 = 512 * (g % 2)
                for k in range(K):
                    rhs = AP(xs.tensor, k * CH,
                             [[G * AW, 128], [AW, gn], [1, CH]])
                    nc.tensor.matmul(ops[0:128, o0:o0 + gn * CH],
                                     wt_sb[:, k * C:(k + 1) * C], rhs,
                                     start=(k == 0), stop=(k == K - 1))
                if g % 2 == 1 or g == NG - 1:
                    ops_tiles.pop(gp)
                    d0 = gp * 2 * G * CH
                    dn = min(2 * G * CH, LOUT - d0)
                    out_sb = o_pool.tile([128, 2 * G * CH], BF16,
                                         name="osb", tag="osb")
                    if dn > G * CH:
                        csrc = AP(ops.tensor, 0,
                                  [[1024, 128], [512, 2], [1, G * CH]])
                    else:
                        csrc = ops[0:128, 0:dn]
                    if gp % 2 == 0:
                        nc.scalar.activation(out_sb[:, 0:dn], csrc,
                                             ACTF.Identity,
                                             bias=bias_sb[:, 0:1])
                    else:
                        nc.vector.tensor_scalar(out_sb[:, 0:dn], csrc,
                                                bias_sb[:, 0:1], None, OP.add)
                    nc.gpsimd.dma_start(outT[b][:, d0:d0 + dn],
                                        out_sb[:, 0:dn])

            seq = [("b", 0), ("b", 1), ("b", 2), ("b", 3), ("w", 0),
                   ("b", 4), ("b", 5), ("w", 1), ("b", 6), ("b", 7),
                   ("w", 2), ("b", 8), ("b", 9), ("w", 3), ("b", 10),
                   ("b", 11), ("w", 4), ("b", 12), ("b", 13), ("w", 5),
                   ("b", 16), ("b", 14), ("w", 6), ("b", 15), ("w", 8),
                   ("w", 7)]
            for kind, idx in seq:
                if kind == "b":
                    stage_b_pair(idx)
                else:
                    stage_w(idx)

    nc.compile()
    _CACHE["nc"] = nc
    return nc


def _make_in_maps(x, offsets, mask, weight, bias):
    x = np.asarray(x, dtype=np.float32)
    offsets = np.asarray(offsets, dtype=np.float32)
    mask = np.asarray(mask, dtype=np.float32)
    weight = np.asarray(weight, dtype=np.float32)
    bias = np.asarray(bias, dtype=np.float32)

    bf16 = ml_dtypes.bfloat16
    # staggered xT: xts[b, p, 128i + c] = x[b, c, 126i + p]
    xts = np.zeros((B, 128, XW), bf16)
    x_bf = x.astype(bf16)
    for i in range(NS):
        t0 = CH * i
        n = min(128, L - t0)
        xts[:, :n, 128 * i:128 * i + C] = x_bf[:, :, t0:t0 + n].transpose(0, 2, 1)
    # wt[c, k*C + o] = weight[o, c, k]
    wt = np.ascontiguousarray(
        weight.transpose(1, 2, 0).reshape(C, K * C).astype(bf16)
    )
    bvec = np.ascontiguousarray(bias.reshape(128, 1))

    # coefficients c[b, t, k, j] = mask * relu(1 - |clip(k + off, 0, 2) - j|)
    off = offsets[:, 0]                                   # [B, LOUT, K]
    p = np.clip(np.arange(K, dtype=np.float32) + off, 0.0, 2.0)
    j = np.arange(3, dtype=np.float32).reshape(1, 1, 1, 3)
    u = np.maximum(0.0, 1.0 - np.abs(p[..., None] - j))   # [B, T, K, 3]
    cf = u * mask.transpose(0, 2, 1)[..., None]           # [B, T, K, 3]

    LP = NS * CH
    cfp = np.zeros((B, LP, K, 3), np.float32)
    cfp[:, :LOUT] = cf
    cfc = cfp.reshape(B, NS, CH, K, 3).astype(bf16)       # [B, NS, tl, K, j]

    # banded tiles A[b, ns, p(128), k, 126]: A[.., tl+j, k, tl] = cfc[.., tl, k, j]
    A = np.zeros((B, NS, 128, K, CH), bf16)
    tl = np.arange(CH)
    for jj in range(3):
        A[:, :, tl + jj, :, tl] = np.moveaxis(cfc[:, :, :, :, jj], 2, 0)

    in_maps = []
    for cid in range(NCORES):
        sl = slice(cid * BPC, (cid + 1) * BPC)
        a_core = A[sl].transpose(0, 2, 1, 3, 4).reshape(BPC, 128, NS * AW)
        in_maps.append({
            "xt_in": np.ascontiguousarray(xts[sl]),
            "a_in": np.ascontiguousarray(a_core),
            "wt": wt,
            "bvec": bvec,
        })
    return in_maps


def kernel(x, offsets, mask, weight, bias):
    nc = _build_program()
    in_maps = _make_in_maps(x, offsets, mask, weight, bias)
    res = bass_utils.run_bass_kernel_spmd(nc, in_maps, core_ids=list(range(NCORES)))
    out = np.empty((B, C, LOUT), np.float32)
    for cid in range(NCORES):
        out[cid * BPC:(cid + 1) * BPC] = res.results[cid]["outT"].astype(np.float32)
    return out


# revision 24
# speedup vs baseline: 1.0630x; 1.0630x over previous
"""Deformable 1D convolution for Trainium2 (8 NeuronCores, data-parallel over batch).

Math (validated against the reference):
    p[t,k]   = clip(k + offsets[b,0,t,k], 0, 2)
    c[k,j,t] = mask[b,k,t] * relu(1 - |p[t,k] - j|)      j in {0,1,2}
    out[b,o,t] = sum_{k,j} c[k,j,t] * (W_k @ x[b])[o, t+j] + bias[o]

v12 "banded-first": both heavy stages run on the PE.

  Host prepares, per chunk i of CH=126 outputs:
    xT_stag[:, 128i:128(i+1)] = x^T rows 126i..126i+128   (staggered x^T)
    A[p, k*126+t] = c[k, p-t, 126i+t] for p-t in {0,1,2}  (banded coeffs, bf16)

  Per chunk (one 128-contraction matmul each):
    MM-B: xs[c, (k,t)] = xT_blk^T @ A        (378 cols; one stationary for all k)
    copy: xs -> bf16 SBUF, split Act/DVE
  Per group of G=4 chunks (stationary W_k amortized over the group):
    MM-W k=0..2: out'[o, (g,t)] += W_k^T @ xs_k[c, (g,t)]  (strided rhs AP)
    bias add (per-partition scalar, out' is [C_out, t]) + bf16 cast,
    alternating Act(Identity+bias) / DVE(tensor_scalar); out DMA per group
    issued from the otherwise-idle GpSimd SWDGE queue.
  Chunks are processed in PAIRS sharing a 2-bank PSUM tile so each Act/DVE
  copy instruction covers two chunks (halves per-instruction overhead).
  Output layout [C_out, t] => host unshard is a concat.
"""

import numpy as np
import ml_dtypes
from contextlib import ExitStack

import concourse.bass as bass
import concourse.mybir as mybir
import concourse.tile as tile
from concourse import bacc
from concourse import bass_utils
from concourse.ap import AP

F32 = mybir.dt.float32
BF16 = mybir.dt.bfloat16
OP = mybir.AluOpType
ACTF = mybir.ActivationFunctionType

B, C, L, K = 16, 128, 4096, 3
LOUT = L - (K - 1)          # 4094
NCORES = 8
BPC = B // NCORES           # batches per core (2)
CH = 126                    # output positions per chunk
NS = -(-LOUT // CH)         # 33 chunks
AW = K * CH                 # A-tile cols per chunk (378)
XW = NS * 128               # staggered xT cols per batch (4224)
G = 4                       # chunks per W-group (one PSUM bank half)
NG = -(-NS // G)            # 9 groups (8 full + 1 single-chunk)

_CACHE = {}


def _build_program():
    if "nc" in _CACHE:
        return _CACHE["nc"]

    nc = bacc.Bacc(
        "TRN2",
        target_bir_lowering=False,
        debug=False,
        enable_asserts=False,
        num_devices=NCORES,
    )

    xt_in = nc.dram_tensor("xt_in", [BPC, 128, XW], BF16, kind="ExternalInput").ap()
    a_in = nc.dram_tensor("a_in", [BPC, 128, NS * AW], BF16,
                          kind="ExternalInput").ap()
    wt = nc.dram_tensor("wt", [C, K * C], BF16, kind="ExternalInput").ap()
    bvec = nc.dram_tensor("bvec", [128, 1], F32, kind="ExternalInput").ap()
    outT = nc.dram_tensor("outT", [BPC, C, LOUT], BF16, kind="ExternalOutput").ap()

    with tile.TileContext(nc) as tc, ExitStack() as ctx:
        const_pool = ctx.enter_context(tc.tile_pool(name="const", bufs=1))
        xt_pool = ctx.enter_context(tc.tile_pool(name="xt", bufs=2))
        a_pool = ctx.enter_context(tc.tile_pool(name="a", bufs=2))
        xs_pool = ctx.enter_context(tc.tile_pool(name="xs", bufs=3))
        o_pool = ctx.enter_context(tc.tile_pool(name="o", bufs=3))
        xps_pool = ctx.enter_context(tc.tile_pool(name="xps", bufs=3, space="PSUM"))
        ops_pool = ctx.enter_context(tc.tile_pool(name="ops", bufs=1, space="PSUM"))

        wt_sb = const_pool.tile([128, K * C], BF16)
        bias_sb = const_pool.tile([128, 1], F32)

        for b in range(BPC):
            # interleave A/xT slices so chunk 0's inputs land first
            xt_sb = xt_pool.tile([128, XW], BF16)
            a_sb = a_pool.tile([128, NS * AW], BF16)
            xt_splits = ((0, 256), (256, 1024), (1024, 2560), (2560, XW))
            a_splits = ((0, 1), (1, 3), (3, 8), (8, 15), (15, 24), (24, NS))
            order = [("a", a_splits[0]), ("x", xt_splits[0]),
                     ("a", a_splits[1]), ("x", xt_splits[1]),
                     ("a", a_splits[2]), ("x", xt_splits[2]),
                     ("a", a_splits[3]), ("x", xt_splits[3]),
                     ("a", a_splits[4]), ("a", a_splits[5])]
            for di, (kind, (s0, s1)) in enumerate(order):
                if kind == "x":
                    nc.sync.dma_start(xt_sb[:, s0:s1], xt_in[b][:, s0:s1])
                else:
                    nc.sync.dma_start(a_sb[:, s0 * AW:s1 * AW],
                                      a_in[b][:, s0 * AW:s1 * AW])
                if b == 0 and di == 1:
                    # constants needed only by MM-W / casts: issue after the
                    # first critical a/xt slices
                    nc.sync.dma_start(wt_sb[:], wt[:])
                    nc.sync.dma_start(bias_sb[:], bvec[:])

            xs_tiles = {}   # group -> xs_sb tile [128, G*AW]
            NPAIR = -(-NS // 2)          # 17 chunk-pairs

            def stage_b_pair(p):
                """MM-B for chunks 2p, 2p+1 into one 2-bank PSUM tile,
                then one paired Act copy + one paired DVE copy."""
                nn = min(2, NS - 2 * p)
                g = (2 * p) // G
                xps = xps_pool.tile([128, 1024], F32, name="xps", tag="xps")
                for ii in range(nn):
                    i = 2 * p + ii
                    nc.tensor.matmul(xps[0:128, 512 * ii:512 * ii + AW],
                                     xt_sb[:, 128 * i:128 * (i + 1)],
                                     a_sb[:, i * AW:(i + 1) * AW],
                                     start=True, stop=True)
                if g not in xs_tiles:
                    xs_tiles[g] = xs_pool.tile([128, G * AW], BF16,
                                               name="xs", tag="xs")
                xs = xs_tiles[g]
                o0 = ((2 * p) % G) * AW
                # paired copies: src strides 512 (psum slots), dst AW (xs slots)
                nc.scalar.activation(
                    AP(xs.tensor, o0, [[G * AW, 128], [AW, nn], [1, 184]]),
                    AP(xps.tensor, 0, [[1024, 128], [512, nn], [1, 184]]),
                    ACTF.Copy)
                nc.vector.tensor_copy(
                    AP(xs.tensor, o0 + 184, [[G * AW, 128], [AW, nn], [1, 194]]),
                    AP(xps.tensor, 184, [[1024, 128], [512, nn], [1, 194]]))

            ops_tiles = {}

            def stage_w(g):
                """W-contraction for group g into its ops-pair half; on the
                pair's second group (or the last), bias+cast + out DMA."""
                gn = min(G, NS - g * G)
                xs = xs_tiles.pop(g)
                gp = g // 2
                if gp not in ops_tiles:
                    ops_tiles[gp] = ops_pool.tile([128, 1024], F32,
                                                  name="ops", tag="ops")
                ops = ops_tiles[gp]
                o0 = 512 * (g % 2)
                for k in range(K):
                    rhs = AP(xs.tensor, k * CH,
                             [[G * AW, 128], [AW, gn], [1, CH]])
                    nc.tensor.matmul(ops[0:128, o0:o0 + gn * CH],
                                     wt_sb[:, k * C:(k + 1) * C], rhs,
                                     start=(k == 0), stop=(k == K - 1))
                if g % 2 == 1 or g == NG - 1:
                    ops_tiles.pop(gp)
                    d0 = gp * 2 * G * CH
                    dn = min(2 * G * CH, LOUT - d0)
                    out_sb = o_pool.tile([128, 2 * G * CH], BF16,
                                         name="osb", tag="osb")
                    if dn > G * CH:
                        csrc = AP(ops.tensor, 0,
                                  [[1024, 128], [512, 2], [1, G * CH]])
                    else:
                        csrc = ops[0:128, 0:dn]
                    if gp % 2 == 0:
                        nc.scalar.activation(out_sb[:, 0:dn], csrc,
                                             ACTF.Identity,
                                             bias=bias_sb[:, 0:1])
                    else:
                        nc.vector.tensor_scalar(out_sb[:, 0:dn], csrc,
                                                bias_sb[:, 0:1], None, OP.add)
                    nc.gpsimd.dma_start(outT[b][:, d0:d0 + dn],
                                        out_sb[:, 0:dn])

            seq = [("b", 0), ("b", 1), ("b", 2), ("b", 3), ("w", 0),
                   ("b", 4), ("b", 5), ("w", 1), ("b", 6), ("b", 7),
                   ("w", 2), ("b", 8), ("b", 9), ("w", 3), ("b", 10),
                   ("b", 11), ("w", 4), ("b", 12), ("b", 13), ("w", 5),
                   ("b", 14), ("b", 15), ("w", 6), ("b", 16), ("w", 7),
                   ("w", 8)]
            for kind, idx in seq:
                if kind == "b":
                    stage_b_pair(idx)
                else:
                    stage_w(idx)

    nc.compile()
    _CACHE["nc"] = nc
    return nc


def _make_in_maps(x, offsets, mask, weight, bias):
    x = np.asarray(x, dtype=np.float32)
    offsets = np.asarray(offsets, dtype=np.float32)
    mask = np.asarray(mask, dtype=np.float32)
    weight = np.asarray(weight, dtype=np.float32)
    bias = np.asarray(bias, dtype=np.float32)

    bf16 = ml_dtypes.bfloat16
    # staggered xT: xts[b, p, 128i + c] = x[b, c, 126i + p]
    xts = np.zeros((B, 128, XW), bf16)
    x_bf = x.astype(bf16)
    for i in range(NS):
        t0 = CH * i
        n = min(128, L - t0)
        xts[:, :n, 128 * i:128 * i + C] = x_bf[:, :, t0:t0 + n].transpose(0, 2, 1)
    # wt[c, k*C + o] = weight[o, c, k]
    wt = np.ascontiguousarray(
        weight.transpose(1, 2, 0).reshape(C, K * C).astype(bf16)
    )
    bvec = np.ascontiguousarray(bias.reshape(128, 1))

    # coefficients c[b, t, k, j] = mask * relu(1 - |clip(k + off, 0, 2) - j|)
    off = offsets[:, 0]                                   # [B, LOUT, K]
    p = np.clip(np.arange(K, dtype=np.float32) + off, 0.0, 2.0)
    j = np.arange(3, dtype=np.float32).reshape(1, 1, 1, 3)
    u = np.maximum(0.0, 1.0 - np.abs(p[..., None] - j))   # [B, T, K, 3]
    cf = u * mask.transpose(0, 2, 1)[..., None]           # [B, T, K, 3]

    LP = NS * CH
    cfp = np.zeros((B, LP, K, 3), np.float32)
    cfp[:, :LOUT] = cf
    cfc = cfp.reshape(B, NS, CH, K, 3).astype(bf16)       # [B, NS, tl, K, j]

    # banded tiles A[b, ns, p(128), k, 126]: A[.., tl+j, k, tl] = cfc[.., tl, k, j]
    A = np.zeros((B, NS, 128, K, CH), bf16)
    tl = np.arange(CH)
    for jj in range(3):
        A[:, :, tl + jj, :, tl] = np.moveaxis(cfc[:, :, :, :, jj], 2, 0)

    in_maps = []
    for cid in range(NCORES):
        sl = slice(cid * BPC, (cid + 1) * BPC)
        a_core = A[sl].transpose(0, 2, 1, 3, 4).reshape(BPC, 128, NS * AW)
        in_maps.append({
            "xt_in": np.ascontiguousarray(xts[sl]),
            "a_in": np.ascontiguousarray(a_core),
            "wt": wt,
            "bvec": bvec,
        })
    return in_maps


def kernel(x, offsets, mask, weight, bias):
    nc = _build_program()
    in_maps = _make_in_maps(x, offsets, mask, weight, bias)
    res = bass_utils.run_bass_kernel_spmd(nc, in_maps, core_ids=list(range(NCORES)))
    out = np.empty((B, C, LOUT), np.float32)
    for cid in range(NCORES):
        out[cid * BPC:(cid + 1) * BPC] = res.results[cid]["outT"].astype(np.float32)
    return out
